# revision 24
# baseline (speedup 1.0000x reference)
"""Trainium2 Bass kernel for AttnDecoderRNN single step (GRU + masked attention + MLP/BN).

Sharding: data-parallel over batch B=256 across 8 cores (32 rows each).
Weights replicated per core, pre-transposed on host so every matmul runs in
[dim, batch] layout on the PE. Single pass over encoder_outputs:
  - energies via DVE multiply + ACT accumulate against a PE-broadcast query
  - exp without max subtraction (query is L2-normalized, energies are small)
  - context accumulated on PE with enc tiles as stationary weights and the
    exp-weights as a 1-column moving operand
BatchNorm batch stats cross 8 cores via a small DRAM AllReduce.

I/O is packed into few tensors (the PJRT dispatch path pays a large fixed
cost per buffer): inputs = enc/wgi/wgh/awt/w1t + one packed const tensor,
output = one packed [128, 672] tensor.
"""

import sys

import numpy as np

for _p in ("/opt/trn_rl_repo", "/root/.axon_site/_ro/trn_rl_repo"):
    if _p not in sys.path:
        sys.path.append(_p)

from concourse import bacc, bass_utils, mybir, tile  # noqa: E402

H = 1024
P = 3
S = 512
B = 256
NCORES = 8
BS = B // NCORES  # 32
EPS = 1e-5
NEG = -1.0e30
SC = 4  # seq chunks of 128
GI = 4  # batch rows per inner group
IGN = BS // GI  # 8 groups
FP = mybir.dt.float32
F4 = np.float32

# packed const tensor column layout
_C_OFF = {}
_c = 0
for _name, _w in [
    ("lctx", 8 * BS),
    ("h0", 8 * BS),
    ("bgi", 24 * BS),
    ("bhhn", 8),
    ("ab", 8),
    ("b1", 8),
    ("gamma", 8),
    ("beta", 8),
    ("w2", 24),
    ("b2", 1),
    ("mask", 128),
    ("ones", 128),
    ("ident", 128),
]:
    _C_OFF[_name] = (_c, _w)
    _c += _w
CC = _c

# packed output column layout: ht 0:256, ctxt 256:512, wt 512:640, outt [0:3, 640:672]
OUT_HT = 0
OUT_CTX = 8 * BS
OUT_WT = 16 * BS
OUT_OT = 16 * BS + 128
OUT_W = OUT_OT + BS


def _build(ncores=NCORES, use_collective=True):
    A = mybir.AluOpType
    AF = mybir.ActivationFunctionType

    nc = bacc.Bacc(
        "TRN2",
        target_bir_lowering=False,
        debug=False,
        enable_asserts=False,
        num_devices=ncores,
    )

    def inp(name, shape):
        return nc.dram_tensor(name, shape, FP, kind="ExternalInput").ap()

    enc = inp("enc", [S, BS, H])
    wgi = inp("wgi", [H, 3 * H])
    wgh = inp("wgh", [H, 3 * H])
    awt = inp("awt", [H, H])
    w1t = inp("w1t", [2 * H, H])
    cst = inp("cst", [128, CC])
    res_o = nc.dram_tensor("res", [128, OUT_W], FP, kind="ExternalOutput").ap()

    def col(jc):
        return slice(jc * BS, (jc + 1) * BS)

    with tile.TileContext(nc) as tc:
        from contextlib import ExitStack

        with ExitStack() as ctx:
            cpool = ctx.enter_context(tc.tile_pool(name="consts", bufs=1))
            wpool = ctx.enter_context(tc.tile_pool(name="weights", bufs=2))
            encpool = ctx.enter_context(tc.tile_pool(name="enc", bufs=4))
            spool = ctx.enter_context(tc.tile_pool(name="small", bufs=4))
            bnpool = ctx.enter_context(tc.tile_pool(name="bn", bufs=24))
            scpool = ctx.enter_context(tc.tile_pool(name="scratch", bufs=2))
            qbpool = ctx.enter_context(tc.tile_pool(name="qb", bufs=4))
            ppool = ctx.enter_context(tc.tile_pool(name="psum", bufs=2, space="PSUM"))
            p2pool = ctx.enter_context(tc.tile_pool(name="psum2", bufs=2, space="PSUM"))
            dpool = ctx.enter_context(tc.tile_pool(name="dram", bufs=1, space="DRAM"))

            # ---- packed consts into SBUF ----
            cst_sb = cpool.tile([128, CC], FP, tag="c_all")
            nc.sync.dma_start(cst_sb[:], cst[:])

            def cs(name):
                off, w = _C_OFF[name]
                return cst_sb[:, off : off + w]

            lctx_sb = cs("lctx")
            h0_sb = cs("h0")
            bgi_sb = cs("bgi")
            bhhn_sb = cs("bhhn")
            ab_sb = cs("ab")
            b1_sb = cs("b1")
            gam_sb = cs("gamma")
            bet_sb = cs("beta")
            w2_sb = cs("w2")
            b2_sb = cst_sb[0:3, _C_OFF["b2"][0] : _C_OFF["b2"][0] + 1]
            mask_sb = cs("mask")
            ones_sb = cs("ones")
            ident_sb = cs("ident")

            # ---- GRU: gi/gh in transposed layout [j, i] ----
            gi_ps = ppool.tile([128, 24 * BS], FP, tag="acc")
            gh_ps = ppool.tile([128, 24 * BS], FP, tag="acc")
            for dc in range(8):
                sl_i = wpool.tile([128, 3 * H], FP, tag="gslab_i")
                nc.sync.dma_start(sl_i[:], wgi[dc * 128 : (dc + 1) * 128, :])
                sl_h = wpool.tile([128, 3 * H], FP, tag="gslab_h")
                nc.sync.dma_start(sl_h[:], wgh[dc * 128 : (dc + 1) * 128, :])
                for jc in range(24):
                    nc.tensor.matmul(
                        gi_ps[:, col(jc)],
                        sl_i[:, jc * 128 : (jc + 1) * 128],
                        lctx_sb[:, col(dc)],
                        start=(dc == 0 and jc % 16 == 0),
                        stop=(dc == 7),
                        skip_group_check=True,
                    )
                    nc.tensor.matmul(
                        gh_ps[:, col(jc)],
                        sl_h[:, jc * 128 : (jc + 1) * 128],
                        h0_sb[:, col(dc)],
                        start=(dc == 0 and jc % 16 == 0),
                        stop=(dc == 7),
                        skip_group_check=True,
                    )

            # ---- gates; h in transposed layout [j, i] ----
            hT_sb = cpool.tile([128, 8 * BS], FP, tag="hT")
            for hc in range(8):
                # r = sigmoid(gi_r + bias_r + gh_r)   (bias_r includes bih+pal+bhh)
                t1 = spool.tile([128, BS], FP, tag="tmp")
                nc.vector.tensor_add(t1[:], gi_ps[:, col(hc)], bgi_sb[:, col(hc)])
                t2 = spool.tile([128, BS], FP, tag="tmp")
                nc.vector.tensor_add(t2[:], t1[:], gh_ps[:, col(hc)])
                r_t = spool.tile([128, BS], FP, tag="gate")
                nc.scalar.activation(r_t[:], t2[:], AF.Sigmoid)
                # z
                t3 = spool.tile([128, BS], FP, tag="tmp")
                nc.vector.tensor_add(t3[:], gi_ps[:, col(8 + hc)], bgi_sb[:, col(8 + hc)])
                t4 = spool.tile([128, BS], FP, tag="tmp")
                nc.vector.tensor_add(t4[:], t3[:], gh_ps[:, col(8 + hc)])
                z_t = spool.tile([128, BS], FP, tag="gate")
                nc.scalar.activation(z_t[:], t4[:], AF.Sigmoid)
                # n = tanh(gi_n + bias_n + r*(gh_n + bhh_n))
                t5 = spool.tile([128, BS], FP, tag="tmp")
                nc.vector.tensor_scalar(
                    t5[:], gh_ps[:, col(16 + hc)], bhhn_sb[:, hc : hc + 1], None, A.add
                )
                t6 = spool.tile([128, BS], FP, tag="tmp")
                nc.vector.tensor_mul(t6[:], r_t[:], t5[:])
                t7 = spool.tile([128, BS], FP, tag="tmp")
                nc.vector.tensor_add(t7[:], gi_ps[:, col(16 + hc)], bgi_sb[:, col(16 + hc)])
                t8 = spool.tile([128, BS], FP, tag="tmp")
                nc.vector.tensor_add(t8[:], t7[:], t6[:])
                n_t = spool.tile([128, BS], FP, tag="gate")
                nc.scalar.activation(n_t[:], t8[:], AF.Tanh)
                # h = n + z*(h0 - n)
                t9 = spool.tile([128, BS], FP, tag="tmp")
                nc.vector.tensor_sub(t9[:], h0_sb[:, col(hc)], n_t[:])
                t10 = spool.tile([128, BS], FP, tag="tmp")
                nc.vector.tensor_mul(t10[:], z_t[:], t9[:])
                nc.vector.tensor_add(hT_sb[:, col(hc)], n_t[:], t10[:])
            nc.sync.dma_start(res_o[:, OUT_HT : OUT_HT + 8 * BS], hT_sb[:])

            # ---- attention query q = attn_W @ h + b (transposed layout) ----
            q_ps = ppool.tile([128, 8 * BS], FP, tag="acc")
            for dc in range(8):
                asl = wpool.tile([128, H], FP, tag="aslab")
                nc.sync.dma_start(asl[:], awt[dc * 128 : (dc + 1) * 128, :])
                for jc in range(8):
                    nc.tensor.matmul(
                        q_ps[:, col(jc)],
                        asl[:, jc * 128 : (jc + 1) * 128],
                        hT_sb[:, col(dc)],
                        start=(dc == 0 and jc == 0),
                        stop=(dc == 7),
                        skip_group_check=True,
                    )
            q_sb = cpool.tile([128, 8 * BS], FP, tag="q")
            ssq_ps = ppool.tile([32, 1], FP, tag="acc")
            for jc in range(8):
                nc.scalar.activation(
                    q_sb[:, col(jc)], q_ps[:, col(jc)], AF.Identity, bias=ab_sb[:, jc : jc + 1]
                )
                sq = spool.tile([128, BS], FP, tag="tmp")
                nc.scalar.activation(sq[:], q_sb[:, col(jc)], AF.Square)
                nc.tensor.matmul(
                    ssq_ps[:], sq[:], ones_sb[:, 0:1], start=(jc == 0), stop=(jc == 7)
                )

            # q as rows [32, 1024] via PE transpose
            qrows = cpool.tile([32, H], FP, tag="qrows")
            for jc in range(8):
                qr_ps = p2pool.tile([128, 512], FP, tag="sc")
                nc.tensor.transpose(qr_ps[0:32, 0:128], q_sb[:, col(jc)], ident_sb[:])
                nc.scalar.activation(
                    qrows[0:32, jc * 128 : (jc + 1) * 128], qr_ps[0:32, 0:128], AF.Copy
                )

            # sinv = 1/||q|| broadcast to [128, 32]
            sroot = spool.tile([32, 1], FP, tag="misc")
            nc.scalar.activation(sroot[:], ssq_ps[:], AF.Sqrt)
            srow_ps = p2pool.tile([128, 512], FP, tag="sc")
            nc.tensor.transpose(srow_ps[0:1, 0:32], sroot[:], ident_sb[0:32, 0:32])
            sinvrow = spool.tile([1, 32], FP, tag="misc")
            nc.vector.reciprocal(sinvrow[:], srow_ps[0:1, 0:32])
            sb_ps = p2pool.tile([128, 512], FP, tag="sc")
            nc.tensor.matmul(
                sb_ps[0:128, 0:32], ones_sb[0:1, :], sinvrow[:], start=True, stop=True
            )
            sinvb_sb = cpool.tile([128, 32], FP, tag="sinvb")
            nc.scalar.activation(sinvb_sb[:], sb_ps[0:128, 0:32], AF.Copy)

            # ---- attention main loop: single pass over enc ----
            E_sb = cpool.tile([128, 128], FP, tag="E")
            wexp_sb = cpool.tile([128, 128], FP, tag="wexp")
            ctx_ps = ppool.tile([128, 8 * BS], FP, tag="acc")
            z_ps = ppool.tile([1, 128], FP, tag="acc")

            for ig in range(IGN):
                qbs = []
                for il in range(GI):
                    i = ig * GI + il
                    qb_sb = qbpool.tile([128, H], FP, tag="qb")
                    for hh in range(2):
                        qb_ps = p2pool.tile([128, 512], FP, tag="sc")
                        nc.tensor.matmul(
                            qb_ps[:],
                            ident_sb[0:32, i : i + 1].to_broadcast((32, 128)),
                            qrows[:, hh * 512 : (hh + 1) * 512],
                            start=True,
                            stop=True,
                        )
                        nc.scalar.activation(
                            qb_sb[:, hh * 512 : (hh + 1) * 512], qb_ps[:], AF.Copy
                        )
                    qbs.append(qb_sb)

                for c in range(SC):
                    et = encpool.tile([128, GI, H], FP, tag="enc")
                    nc.sync.dma_start(
                        et[:], enc[c * 128 : (c + 1) * 128, ig * GI : (ig + 1) * GI, :]
                    )
                    base = c * 32 + ig * GI
                    for il in range(GI):
                        i = ig * GI + il
                        scr = scpool.tile([128, H], FP, tag="ttr")
                        nc.vector.tensor_mul(scr[:], et[:, il, :], qbs[il][:])
                        nc.scalar.activation(
                            scr[:],
                            scr[:],
                            AF.Identity,
                            accum_out=E_sb[:, c * 32 + i : c * 32 + i + 1],
                        )
                    # scale by 1/||q||, add mask, exp
                    m1 = spool.tile([128, GI], FP, tag="mtmp")
                    nc.vector.tensor_mul(
                        m1[:], E_sb[:, base : base + GI], sinvb_sb[:, ig * GI : (ig + 1) * GI]
                    )
                    m2 = spool.tile([128, GI], FP, tag="mtmp")
                    nc.vector.tensor_add(m2[:], m1[:], mask_sb[:, base : base + GI])
                    nc.scalar.activation(wexp_sb[:, base : base + GI], m2[:], AF.Exp)
                    # Z partial sums (per (c, ig) column range, summed later)
                    nc.tensor.matmul(
                        z_ps[0:1, base : base + GI],
                        ones_sb[:, 0:1],
                        wexp_sb[:, base : base + GI],
                        start=True,
                        stop=True,
                    )
                    # context accumulation: enc tile as weights, wexp column moving
                    for il in range(GI):
                        i = ig * GI + il
                        for hc in range(8):
                            nc.tensor.matmul(
                                ctx_ps[:, hc * 32 + i : hc * 32 + i + 1],
                                et[:, il, hc * 128 : (hc + 1) * 128],
                                wexp_sb[:, c * 32 + i : c * 32 + i + 1],
                                start=(ig == 0 and c == 0 and il == 0 and hc == 0),
                                stop=(c == SC - 1),
                                skip_group_check=True,
                            )

            # ---- zinv = 1/Z broadcast ----
            zrow_sb = spool.tile([1, 128], FP, tag="misc")
            nc.scalar.activation(zrow_sb[:], z_ps[:], AF.Copy)
            za = spool.tile([1, 32], FP, tag="misc")
            nc.vector.tensor_add(za[:], zrow_sb[0:1, 0:32], zrow_sb[0:1, 32:64])
            zb = spool.tile([1, 32], FP, tag="misc")
            nc.vector.tensor_add(zb[:], zrow_sb[0:1, 64:96], zrow_sb[0:1, 96:128])
            zs = spool.tile([1, 32], FP, tag="misc")
            nc.vector.tensor_add(zs[:], za[:], zb[:])
            zinvrow = spool.tile([1, 32], FP, tag="misc")
            nc.vector.reciprocal(zinvrow[:], zs[:])
            zb_ps = p2pool.tile([128, 512], FP, tag="sc")
            nc.tensor.matmul(
                zb_ps[0:128, 0:32], ones_sb[0:1, :], zinvrow[:], start=True, stop=True
            )
            zinvb_sb = cpool.tile([128, 32], FP, tag="zinvb")
            nc.scalar.activation(zinvb_sb[:], zb_ps[0:128, 0:32], AF.Copy)

            # ---- normalize context and weights, write out ----
            ctxT_sb = cpool.tile([128, 8 * BS], FP, tag="ctxT")
            for hc in range(8):
                nc.vector.tensor_mul(ctxT_sb[:, col(hc)], ctx_ps[:, col(hc)], zinvb_sb[:])
            nc.sync.dma_start(res_o[:, OUT_CTX : OUT_CTX + 8 * BS], ctxT_sb[:])
            wn_sb = cpool.tile([128, 128], FP, tag="wn")
            for c in range(SC):
                nc.vector.tensor_mul(
                    wn_sb[:, c * 32 : (c + 1) * 32], wexp_sb[:, c * 32 : (c + 1) * 32], zinvb_sb[:]
                )
            nc.sync.dma_start(res_o[:, OUT_WT : OUT_WT + 128], wn_sb[:])

            # ---- MLP: y = relu(W1 @ [h; ctx] + b1) in transposed layout ----
            y_ps = ppool.tile([128, 8 * BS], FP, tag="acc")
            for dc in range(16):
                wsl = wpool.tile([128, H], FP, tag="w1slab")
                nc.sync.dma_start(wsl[:], w1t[dc * 128 : (dc + 1) * 128, :])
                rhs = hT_sb[:, col(dc)] if dc < 8 else ctxT_sb[:, col(dc - 8)]
                for jc in range(8):
                    nc.tensor.matmul(
                        y_ps[:, col(jc)],
                        wsl[:, jc * 128 : (jc + 1) * 128],
                        rhs,
                        start=(dc == 0 and jc == 0),
                        stop=(dc == 15),
                        skip_group_check=True,
                    )
            yT_sb = cpool.tile([128, 8 * BS], FP, tag="yT")
            s12_sb = cpool.tile([128, 16], FP, tag="s12")
            for jc in range(8):
                nc.scalar.activation(
                    yT_sb[:, col(jc)], y_ps[:, col(jc)], AF.Relu, bias=b1_sb[:, jc : jc + 1]
                )
                sc1 = spool.tile([128, BS], FP, tag="tmp")
                nc.scalar.activation(
                    sc1[:], yT_sb[:, col(jc)], AF.Identity, accum_out=s12_sb[:, jc : jc + 1]
                )
                sc2 = spool.tile([128, BS], FP, tag="tmp")
                nc.scalar.activation(
                    sc2[:], yT_sb[:, col(jc)], AF.Square, accum_out=s12_sb[:, 8 + jc : 9 + jc]
                )

            # ---- cross-core reduction of BN stats ----
            if ncores > 1 and use_collective:
                cc_in = dpool.tile([128, 16], FP, tag="ccin")
                cc_out = dpool.tile([128, 16], FP, tag="ccout")
                nc.sync.dma_start(cc_in[:], s12_sb[:])
                nc.gpsimd.collective_compute(
                    "AllReduce",
                    A.add,
                    replica_groups=[list(range(ncores))],
                    ins=[cc_in.opt()],
                    outs=[cc_out.opt()],
                )
                s12r_sb = cpool.tile([128, 16], FP, tag="s12r")
                nc.sync.dma_start(s12r_sb[:], cc_out[:])
                nb = float(B)
            else:
                s12r_sb = s12_sb
                nb = float(BS)

            # ---- BN + final linear ----
            eps_sb = cpool.tile([128, 1], FP, tag="eps")
            nc.vector.memset(eps_sb[:], EPS)
            out_ps = ppool.tile([3, BS], FP, tag="acc")
            for jc in range(8):
                mu = bnpool.tile([128, 1], FP, tag="bn")
                nc.scalar.activation(mu[:], s12r_sb[:, jc : jc + 1], AF.Copy, scale=1.0 / nb)
                ey2 = bnpool.tile([128, 1], FP, tag="bn")
                nc.scalar.activation(
                    ey2[:], s12r_sb[:, 8 + jc : 9 + jc], AF.Copy, scale=1.0 / nb
                )
                mu2 = bnpool.tile([128, 1], FP, tag="bn")
                nc.vector.tensor_mul(mu2[:], mu[:], mu[:])
                var = bnpool.tile([128, 1], FP, tag="bn")
                nc.vector.tensor_sub(var[:], ey2[:], mu2[:])
                sd = bnpool.tile([128, 1], FP, tag="bn")
                nc.scalar.activation(sd[:], var[:], AF.Sqrt, bias=eps_sb[:])
                rinv = bnpool.tile([128, 1], FP, tag="bn")
                nc.vector.reciprocal(rinv[:], sd[:])
                scl = bnpool.tile([128, 1], FP, tag="bn")
                nc.vector.tensor_mul(scl[:], gam_sb[:, jc : jc + 1], rinv[:])
                msc = bnpool.tile([128, 1], FP, tag="bn")
                nc.vector.tensor_mul(msc[:], mu[:], scl[:])
                sh = bnpool.tile([128, 1], FP, tag="bn")
                nc.vector.tensor_sub(sh[:], bet_sb[:, jc : jc + 1], msc[:])
                yn = spool.tile([128, BS], FP, tag="yn")
                nc.vector.tensor_scalar(yn[:], yT_sb[:, col(jc)], scl[:], sh[:], A.mult, A.add)
                nc.tensor.matmul(
                    out_ps[:],
                    w2_sb[:, jc * 3 : (jc + 1) * 3],
                    yn[:],
                    start=(jc == 0),
                    stop=(jc == 7),
                )
            outT_sb = spool.tile([3, BS], FP, tag="ot")
            nc.scalar.activation(outT_sb[:], out_ps[:], AF.Identity, bias=b2_sb)
            nc.sync.dma_start(res_o[0:3, OUT_OT : OUT_OT + BS], outT_sb[:])

    nc.compile()
    return nc


def _t128(x):
    """[D, C] -> [128, (D//128)*C] tiled so row d = a*128+p lands at [p, a*C + c]."""
    x = np.ascontiguousarray(x, dtype=F4)
    D, C = x.shape
    a = D // 128
    return np.ascontiguousarray(x.reshape(a, 128, C).transpose(1, 0, 2).reshape(128, a * C))


def _unt128(x, D, C):
    """Inverse of _t128: [128, (D//128)*C] -> [D, C]."""
    a = D // 128
    return x.reshape(128, a, C).transpose(1, 0, 2).reshape(D, C)


def prepare_in_maps(inputs):
    ins = {k: np.asarray(v) for k, v in inputs.items()}
    pal = np.asarray(ins["palette"], F4)[0]  # [B, 3]
    lctx = np.asarray(ins["last_context"], F4)[0]  # [B, H]
    h0 = np.asarray(ins["last_hidden"], F4)[0]  # [B, H]
    enc = np.asarray(ins["encoder_outputs"], F4)  # [S, B, H]
    sizes = np.asarray(ins["each_input_size"]).astype(np.int64)  # [B]
    Wih = np.asarray(ins["gru_Wih"], F4)
    Whh = np.asarray(ins["gru_Whh"], F4)
    bih = np.asarray(ins["gru_bih"], F4)
    bhh = np.asarray(ins["gru_bhh"], F4)
    aW = np.asarray(ins["attn_W"], F4)
    ab_ = np.asarray(ins["attn_b"], F4)
    W1 = np.asarray(ins["out_W1"], F4)
    b1 = np.asarray(ins["out_b1"], F4)
    gam = np.asarray(ins["bn_gamma"], F4)
    bet = np.asarray(ins["bn_beta"], F4)
    W2 = np.asarray(ins["out_W2"], F4)
    b2 = np.asarray(ins["out_b2"], F4)

    shared = {
        "wgi": np.ascontiguousarray(Wih[:, 3:].T, F4),
        "wgh": np.ascontiguousarray(Whh.T, F4),
        "awt": np.ascontiguousarray(aW.T, F4),
        "w1t": np.ascontiguousarray(W1.T, F4),
    }
    cst_shared = np.zeros((128, CC), F4)

    def put(name, arr):
        off, w = _C_OFF[name]
        cst_shared[: arr.shape[0], off : off + w] = arr

    put("bhhn", _t128(bhh[2 * H :][:, None]))
    put("ab", _t128(ab_[:, None]))
    put("b1", _t128(b1[:, None]))
    put("gamma", _t128(gam[:, None]))
    put("beta", _t128(bet[:, None]))
    put("w2", _t128(np.ascontiguousarray(W2.T)))
    put("b2", b2[:, None])
    put("ones", np.ones((128, 128), F4))
    put("ident", np.eye(128, dtype=F4))

    # gi bias: palette part + bih; fold bhh into r,z gate biases
    bias_gi = pal @ Wih[:, :3].T + bih  # [B, 3H]
    bias_gi[:, : 2 * H] += bhh[: 2 * H]
    mask_full = np.where(
        np.arange(S)[:, None] < sizes[None, :], np.float32(0.0), np.float32(NEG)
    ).astype(F4)  # [S, B]

    in_maps = []
    for c in range(NCORES):
        i0 = c * BS
        m = dict(shared)
        cstc = cst_shared.copy()

        def putc(name, arr):
            off, w = _C_OFF[name]
            cstc[:, off : off + w] = arr

        putc("lctx", _t128(lctx[i0 : i0 + BS].T))
        putc("h0", _t128(h0[i0 : i0 + BS].T))
        putc("bgi", _t128(bias_gi[i0 : i0 + BS].T))
        putc("mask", _t128(mask_full[:, i0 : i0 + BS]))
        m["cst"] = cstc
        m["enc"] = np.ascontiguousarray(enc[:, i0 : i0 + BS, :])
        in_maps.append(m)
    return in_maps


def assemble_outputs(results):
    outs, ctxs, hs, ws = [], [], [], []
    for r in results:
        res = r["res"]
        hs.append(_unt128(res[:, OUT_HT : OUT_HT + 8 * BS], H, BS).T)
        ctxs.append(_unt128(res[:, OUT_CTX : OUT_CTX + 8 * BS], H, BS).T)
        ws.append(_unt128(res[:, OUT_WT : OUT_WT + 128], S, BS).T)
        outs.append(res[0:3, OUT_OT : OUT_OT + BS].T)
    out = np.ascontiguousarray(np.concatenate(outs, 0), F4)
    context = np.ascontiguousarray(np.concatenate(ctxs, 0), F4)[None]
    h = np.ascontiguousarray(np.concatenate(hs, 0), F4)[None]
    attn = np.ascontiguousarray(np.concatenate(ws, 0), F4)[:, None, :]
    return out, context, h, attn


_BUILT = None


def get_built():
    global _BUILT
    if _BUILT is None:
        _BUILT = _build()
    return _BUILT


def run(inputs, trace=False, **kw):
    nc = get_built()
    in_maps = prepare_in_maps(inputs)
    res = bass_utils.run_bass_kernel_spmd(
        nc, in_maps, core_ids=list(range(NCORES)), trace=trace, **kw
    )
    return assemble_outputs(res.results), res


def kernel(**inputs):
    outputs, _ = run(inputs)
    return outputs
